# revision 42
# baseline (speedup 1.0000x reference)
# Banded (sliding-window) multi-head attention for Trainium2, 8 NeuronCores.
# Sharding: batch x head-group (2 batches x 4 groups of 4 heads). QKV is
# column-parallel, W_o row-parallel (megatron TP); host sums the 4 partial
# outputs per batch and adds b_o.
#
# QKV projection runs as a 3-term error-compensated fp8(e4m3) product in
# DoubleRow perf mode (2 k-tiles per pass, 0.5 cyc/row):
#   4096*x*W ~= A@P + B@(P/16) + A@S2
# with A=e4m3(16x), B=e4m3(256(x-A/16)), P=e4m3(256W), S2=e4m3(256W-P).
# Scores/PV/O-proj stay bf16. PV is computed transposed (out [q,d], 128
# query partitions) and the per-head outputs are PE-transposed back for a
# row-parallel bf16 O-projection. All rescales are exact powers of two.
import math
import numpy as np
import ml_dtypes

import concourse.bass as bass
import concourse.tile as tile
import concourse.mybir as mybir
import concourse.bass_utils as bass_utils

F32 = mybir.dt.float32
BF16 = mybir.dt.bfloat16
FP8 = mybir.dt.float8e4
BF16NP = ml_dtypes.bfloat16
E4NP = ml_dtypes.float8_e4m3
DR = mybir.MatmulPerfMode.DoubleRow

B, S, DIN, EMB, NH, WIN = 2, 4096, 1024, 1024, 16, 256
HD = EMB // NH          # 64
W = WIN // 2            # 128 one-sided window
HL = 4                  # heads per core
NKC = S // 128          # 32 key chunks
NQC = S // 128          # 32 query chunks
NEGB = -1e15
# scores arrive at 4096^2 x true scale; exp(score * SCALE_EXP) folds both
# the 1/sqrt(hd) and the fp8 staging scales (all powers of two, exact).
SCALE_EXP = (1.0 / math.sqrt(HD)) / (4096.0 * 4096.0)

_MAX_WAITS = 1


def _split_excess_waits(nc):
    # This walrus build accepts a single sync-wait per instruction; move
    # excess waits onto same-engine NoOps inserted immediately before.
    for bb in nc.m.functions[0].blocks:
        new_insts = []
        changed = False
        for inst in bb.instructions:
            si = inst.sync_info
            if si is not None and si.on_wait and len(si.on_wait) > _MAX_WAITS:
                waits = list(si.on_wait)
                head, tail = waits[:-_MAX_WAITS], waits[-_MAX_WAITS:]
                k = 0
                while head:
                    chunk, head = head[:_MAX_WAITS], head[_MAX_WAITS:]
                    new_insts.append(mybir.InstNoOp(
                        name=f"{inst.name}-ws{k}", engine=inst.engine,
                        ins=[], outs=[],
                        sync_info=mybir.SyncInfo(on_wait=chunk, on_update=[])))
                    k += 1
                inst.sync_info = mybir.SyncInfo(
                    on_wait=tail, on_update=list(si.on_update))
                changed = True
            new_insts.append(inst)
        if changed:
            bb.instructions = new_insts


def _build_program():
    nc = bass.Bass("TRN2", target_bir_lowering=False, debug=False)
    x8d = nc.dram_tensor("x8", [128, 8, 8, 2, 512], FP8, kind="ExternalInput").ap()
    wqk1d = nc.dram_tensor("wqk1", [128, 8, 2, 512], FP8, kind="ExternalInput").ap()
    wqk2d = nc.dram_tensor("wqk2", [128, 4, 2, 512], FP8, kind="ExternalInput").ap()
    wv1d = nc.dram_tensor("wv1", [128, 8, 2, 256], FP8, kind="ExternalInput").ap()
    wv2d = nc.dram_tensor("wv2", [128, 4, 2, 256], FP8, kind="ExternalInput").ap()
    wod = nc.dram_tensor("wo", [128, 2048], BF16, kind="ExternalInput").ap()
    padbd = nc.dram_tensor("padb", [128, NKC], F32, kind="ExternalInput").ap()
    identd = nc.dram_tensor("ident", [128, 128], BF16, kind="ExternalInput").ap()
    y = nc.dram_tensor("y", [S, EMB], BF16, kind="ExternalOutput").ap()

    import contextlib
    with tile.TileContext(nc) as tc, contextlib.ExitStack() as ctx:
        cpool = ctx.enter_context(tc.tile_pool(name="const", bufs=1))
        x8pool = ctx.enter_context(tc.tile_pool(name="x8", bufs=2))
        qkpool = ctx.enter_context(tc.tile_pool(name="qkt", bufs=1))
        vpool = ctx.enter_context(tc.tile_pool(name="v", bufs=1))
        ptpool = ctx.enter_context(tc.tile_pool(name="pt", bufs=72))
        vnpool = ctx.enter_context(tc.tile_pool(name="vn", bufs=8))
        vtpool = ctx.enter_context(tc.tile_pool(name="vt", bufs=1))
        recpool = ctx.enter_context(tc.tile_pool(name="rec", bufs=24))
        ypool = ctx.enter_context(tc.tile_pool(name="ysb", bufs=4))
        pp = ctx.enter_context(tc.tile_pool(name="pp", bufs=2, space="PSUM"))
        stp = ctx.enter_context(tc.tile_pool(name="st", bufs=2, space="PSUM"))
        pvp = ctx.enter_context(tc.tile_pool(name="pv", bufs=2, space="PSUM"))
        yp0 = ctx.enter_context(tc.tile_pool(name="yp0", bufs=1, space="PSUM"))
        yp1 = ctx.enter_context(tc.tile_pool(name="yp1", bufs=1, space="PSUM"))

        # constants; the first x tile and the qk weights are loaded first so
        # the first projection matmuls can start as soon as possible.
        x8_first = x8pool.tile([128, 8, 2, 512], FP8, tag="x8", name="x8_0")
        wqk1_t = cpool.tile([128, 8, 2, 512], FP8, tag="wqk1")
        for h in range(4):
            nc.sync.dma_start(x8_first[:, 2 * h:2 * h + 2],
                              x8d[:, 0, 2 * h:2 * h + 2])
            nc.sync.dma_start(wqk1_t[:, 2 * h:2 * h + 2],
                              wqk1d[:, 2 * h:2 * h + 2])
        wqk2_t = cpool.tile([128, 4, 2, 512], FP8, tag="wqk2")
        nc.sync.dma_start(wqk2_t[:], wqk2d)
        wv1_t = cpool.tile([128, 8, 2, 256], FP8, tag="wv1")
        nc.sync.dma_start(wv1_t[:], wv1d)
        wv2_t = cpool.tile([128, 4, 2, 256], FP8, tag="wv2")
        nc.sync.dma_start(wv2_t[:], wv2d)
        padb_t = cpool.tile([128, NKC], F32, tag="padb")
        nc.sync.dma_start(padb_t[:], padbd)
        ident_t = cpool.tile([128, 128], BF16, tag="ident")
        nc.sync.dma_start(ident_t[:], identd)
        wo_t = cpool.tile([128, 2048], BF16, tag="wo")
        nc.sync.dma_start(wo_t[:], wod)

        # warm the PE p-state during the initial DMA wait: dummy matmuls on
        # a memset tile so the real QKV stream starts at full clock
        warm = cpool.tile([128, 128], BF16, tag="warm")
        nc.vector.memset(warm[:], 0.0)
        warm3 = cpool.tile([128, 384], BF16, tag="warm3")
        nc.vector.memset(warm3[:], 0.0)
        wps = stp.tile([128, 384], F32, tag="st", name="warmps")
        for _ in range(10):
            nc.tensor.matmul(wps[:], warm[:], warm3[:],
                             start=True, stop=True, skip_group_check=True)

        # qT2[p]/kT2[p]: bf16 q^T / k^T for head pair p (partitions
        # 0:64 = head 2p, 64:128 = head 2p+1), at 4096x true scale.
        qT2 = [qkpool.tile([128, S], BF16, tag=f"qt{p}", name=f"qt{p}")
               for p in range(2)]
        kT2 = [qkpool.tile([128, S], BF16, tag=f"kt{p}", name=f"kt{p}")
               for p in range(2)]
        # v_t[kc]: [128 tok, 4 heads, 64 v + ones col], bf16, 4096x scale
        v_t = [vpool.tile([128, HL, 65], BF16, tag=f"v{kc}", name=f"v{kc}")
               for kc in range(NKC)]
        valT = [vtpool.tile([128, S], BF16, tag=f"vt{p}", name=f"valT{p}")
                for p in range(2)]
        pts = {}

        def load_x8(tt):
            if tt == 0:
                return x8_first
            x8_t = x8pool.tile([128, 8, 2, 512], FP8, tag="x8", name=f"x8_{tt}")
            nc.sync.dma_start(x8_t[:], x8d[:, tt])
            return x8_t

        def qk_groups(tt, x8_t):
            # q/k: out [128 cols(2 heads x 64), 512 tok]
            for grp in range(4):
                ps = pp.tile([128, 512], F32, tag="pp", name=f"qk{tt}_{grp}")
                c0 = grp * 128
                for kt in range(8):
                    nc.tensor.matmul(ps[:], wqk1_t[:, kt, :, c0:c0 + 128],
                                     x8_t[:, kt, :, :], start=(kt == 0),
                                     stop=False, perf_mode=DR)
                for g2 in range(4):
                    nc.tensor.matmul(ps[:], wqk2_t[:, g2, :, c0:c0 + 128],
                                     x8_t[:, 2 * g2:2 * g2 + 2, 0, :],
                                     start=False, stop=(g2 == 3), perf_mode=DR)
                dst = qT2[grp] if grp < 2 else kT2[grp - 2]
                nc.vector.tensor_copy(dst[:, tt * 512:(tt + 1) * 512], ps[:])

        def v_group_mm(tt, m, x8_t, ps, half):
            # v: out [128 tok, 4 heads x 64]; emitted in two halves so QK
            # matmuls can interleave between accumulation members.
            ms = m * 128
            if half == 0:
                for kt in range(6):
                    nc.tensor.matmul(ps[:], x8_t[:, kt, :, ms:ms + 128],
                                     wv1_t[:, kt, :, :], start=(kt == 0),
                                     stop=False, perf_mode=DR,
                                     skip_group_check=True)
            else:
                for kt in range(6, 8):
                    nc.tensor.matmul(ps[:], x8_t[:, kt, :, ms:ms + 128],
                                     wv1_t[:, kt, :, :], start=False,
                                     stop=False, perf_mode=DR,
                                     skip_group_check=True)
                for g2 in range(4):
                    nc.tensor.matmul(ps[:], x8_t[:, 2 * g2:2 * g2 + 2, 0, ms:ms + 128],
                                     wv2_t[:, g2, :, :],
                                     start=False, stop=(g2 == 3), perf_mode=DR,
                                     skip_group_check=True)
                kc = tt * 4 + m
                nc.vector.tensor_copy(
                    v_t[kc][:, :, 0:64],
                    ps[:].rearrange("p (h c) -> p h c", h=HL))
                nc.gpsimd.memset(v_t[kc][:, :, 64:65], 1.0)

        def make_pt(j, kc):
            qlo = max(0, 128 * kc - 128)
            qhi = min(S, 128 * kc + 256)
            w = qhi - qlo
            p2, hf = j // 2, (j % 2) * 64
            st = stp.tile([128, w], F32, tag="st", name=f"st{j}_{kc}")
            nc.tensor.matmul(st[:],
                             kT2[p2][hf:hf + 64, kc * 128:(kc + 1) * 128],
                             qT2[p2][hf:hf + 64, qlo:qhi],
                             start=True, stop=True, skip_group_check=True)
            pt = ptpool.tile([128, w], BF16, tag="pt", name=f"pt{j}_{kc}")
            nc.scalar.activation(pt[:], st[:], mybir.ActivationFunctionType.Exp,
                                 bias=padb_t[:, kc:kc + 1], scale=SCALE_EXP)
            # band mask: zero the out-of-band corners (on Pool, SBUF only)
            ms = qlo - 128 * kc + 128
            lo_w = max(0, 128 - ms)
            hi_s = max(0, 256 - ms)
            if lo_w > 0:
                nc.gpsimd.affine_select(
                    out=pt[:, 0:lo_w], in_=pt[:, 0:lo_w],
                    compare_op=mybir.AluOpType.is_ge, fill=0.0,
                    base=ms, pattern=[[1, lo_w]], channel_multiplier=-1)
            if hi_s < w:
                nc.gpsimd.affine_select(
                    out=pt[:, hi_s:w], in_=pt[:, hi_s:w],
                    compare_op=mybir.AluOpType.is_ge, fill=0.0,
                    base=256 - ms - hi_s, pattern=[[-1, w - hi_s]],
                    channel_multiplier=1)
            pts[(j, kc)] = (pt, qlo)

        def batch_kcs(r):
            lo = max(0, 4 * (r - 1) + 1) if r > 0 else 0
            hi = min(NKC, 4 * r + 1)
            return list(range(lo, hi))

        vns = {}

        def pv_pvs(qt, evac_act=False):
            for qc in range(4 * qt, 4 * qt + 4):
                vn = vnpool.tile([128, 256], BF16, tag="vn", name=f"vn{qc}")
                vns[qc] = vn
                for j in range(HL):
                    pv = pvp.tile([128, 65], F32, tag="pv", name=f"pv{j}_{qc}")
                    ks = [kc for kc in (qc - 1, qc, qc + 1) if 0 <= kc < NKC]
                    for i, kc in enumerate(ks):
                        pt, qlo = pts[(j, kc)]
                        off = qc * 128 - qlo
                        nc.tensor.matmul(pv[:], pt[:, off:off + 128],
                                         v_t[kc][:, j, :], start=(i == 0),
                                         stop=(i == len(ks) - 1))
                    rec = recpool.tile([128, 1], F32, tag="rec",
                                       name=f"rec{j}_{qc}")
                    with nc.allow_low_precision(reason="softmax reciprocal"):
                        nc.vector.reciprocal(rec[:], pv[:, 64:65])
                    if evac_act:
                        nc.scalar.activation(
                            vn[:, j * 64:(j + 1) * 64], pv[:, 0:64],
                            mybir.ActivationFunctionType.Copy, scale=rec[:])
                    else:
                        nc.vector.tensor_scalar_mul(
                            vn[:, j * 64:(j + 1) * 64], pv[:, 0:64], rec[:])

        def pv_tps(qt, evac_act=False):
            for qc in range(4 * qt, 4 * qt + 4):
                vn = vns[qc]
                for pair in range(2):
                    tp = pvp.tile([128, 128], BF16, tag="pv",
                                  name=f"tp{pair}_{qc}")
                    nc.tensor.transpose(tp[:], vn[:, pair * 128:(pair + 1) * 128],
                                        ident_t[:])
                    if (evac_act or qt >= 6) and pair == 1:
                        nc.scalar.copy(
                            valT[pair][:, qc * 128:(qc + 1) * 128], tp[:])
                    else:
                        nc.vector.tensor_copy(
                            valT[pair][:, qc * 128:(qc + 1) * 128], tp[:])

        def pv_phase(qt):
            pv_pvs(qt)
            pv_tps(qt)

        def o_chunk(qt, m):
            row0 = (4 * qt + m) * 128
            ysb = ypool.tile([128, 1024], BF16, tag="ysb",
                             name=f"ysb{qt}_{m}")
            for n in range(2):
                ypn = (yp0 if n == 0 else yp1).tile(
                    [128, 512], F32, tag=f"yp{n}", name=f"y{qt}_{m}_{n}")
                for dc in range(2):
                    nc.tensor.matmul(
                        ypn[:], valT[dc][:, row0:row0 + 128],
                        wo_t[:, dc * 1024 + n * 512:dc * 1024 + (n + 1) * 512],
                        start=(dc == 0), stop=(dc == 1),
                        skip_group_check=True)
                if n == 1:
                    nc.scalar.copy(ysb[:, n * 512:(n + 1) * 512], ypn[:])
                else:
                    nc.vector.tensor_copy(
                        ysb[:, n * 512:(n + 1) * 512], ypn[:])
                if qt == 7:
                    nc.sync.dma_start(
                        y[row0:row0 + 128, n * 512:(n + 1) * 512],
                        ysb[:, n * 512:(n + 1) * 512])
            if qt != 7:
                nc.sync.dma_start(y[row0:row0 + 128, :], ysb[:])

        # Round r: QKV projection for token tile r, scores for key chunks
        # <= 4r, attention output for query tile r-2, O-projection for r-3.
        # QK matmuls are interleaved with the v-projection groups so the PE
        # never outruns the Act exp pipeline on the 2-buffer score ring.
        for r in range(11):
            x8_t = load_x8(r) if r < 8 else None
            if r < 8:
                qk_groups(r, x8_t)
            if 2 <= r <= 9:
                pv_phase(r - 2)
            kcs = batch_kcs(r)
            vps = {}
            for i in range(4):
                if x8_t is not None:
                    vps[i] = pp.tile([128, 256], F32, tag="pp",
                                     name=f"v{r}_{i}")
                if i < len(kcs):
                    make_pt(0, kcs[i])
                    make_pt(1, kcs[i])
                if x8_t is not None:
                    v_group_mm(r, i, x8_t, vps[i], 0)
                if i < len(kcs):
                    make_pt(2, kcs[i])
                    make_pt(3, kcs[i])
                if x8_t is not None:
                    v_group_mm(r, i, x8_t, vps[i], 1)
                if 3 <= r <= 10:
                    o_chunk(r - 3, i)

    _split_excess_waits(nc)
    return nc


_NC = {}


def _get_nc(key=0):
    if key not in _NC:
        _NC[key] = _build_program()
    return _NC[key]


def _e4(a):
    return np.clip(a, -240.0, 240.0).astype(E4NP)


def kernel(x, padding_mask, W_qkv, b_qkv, W_o, b_o):
    x = np.asarray(x, np.float32)
    padding_mask = np.asarray(padding_mask)
    W_qkv = np.asarray(W_qkv, np.float32)
    b_qkv = np.asarray(b_qkv, np.float32)
    W_o = np.asarray(W_o, np.float32)
    b_o = np.asarray(b_o, np.float32)

    ident = np.eye(128, dtype=BF16NP)

    in_maps = []
    for c in range(8):
        b, g = c // 4, c % 4
        heads = [g * HL + j for j in range(HL)]
        xT = np.ascontiguousarray(x[b].T)          # [DIN, S]

        # fp8 split of x: A ~ 16x, B ~ 256(x - A/16)
        A = _e4(16.0 * xT)
        Bres = _e4(256.0 * xT - 16.0 * A.astype(np.float32))
        # x8 [128, tt, kt, i, t]
        x8 = np.stack([A, Bres], axis=0).reshape(2, 8, 128, 8, 512)
        x8 = np.ascontiguousarray(x8.transpose(2, 3, 1, 0, 4))

        # packed qk weight cols: [q h0,h1 | q h2,h3 | k h0,h1 | k h2,h3]
        wqk = np.empty((DIN, 512), np.float32)
        wv = np.empty((DIN, 256), np.float32)
        wo = np.empty((256, EMB), np.float32)
        for j, h in enumerate(heads):
            base = h * 192
            qcol = 128 * (j // 2) + 64 * (j % 2)
            wqk[:, qcol:qcol + 64] = W_qkv[:, base:base + 64]
            wqk[:, 256 + qcol:256 + qcol + 64] = W_qkv[:, base + 64:base + 128]
            wv[:, j * 64:(j + 1) * 64] = W_qkv[:, base + 128:base + 192]
            wo[j * 64:(j + 1) * 64, :] = W_o[h * 64:(h + 1) * 64, :]

        def split_w(Wm):
            P = _e4(256.0 * Wm)
            P16 = _e4(P.astype(np.float32) / 16.0)
            S2 = _e4(256.0 * Wm - P.astype(np.float32))
            return P, P16, S2

        P, P16, S2 = split_w(wqk)
        ncols = 512
        w1 = np.stack([P, P16], axis=0).reshape(2, 8, 128, ncols)
        w1 = np.ascontiguousarray(w1.transpose(2, 1, 0, 3))   # [128, kt, i, c]
        w2 = np.ascontiguousarray(
            S2.reshape(4, 2, 128, ncols).transpose(2, 0, 1, 3))
        Pv, Pv16, S2v = split_w(wv)
        wv1 = np.stack([Pv, Pv16], axis=0).reshape(2, 8, 128, 256)
        wv1 = np.ascontiguousarray(wv1.transpose(2, 1, 0, 3))
        wv2 = np.ascontiguousarray(
            S2v.reshape(4, 2, 128, 256).transpose(2, 0, 1, 3))

        wo8 = np.ascontiguousarray(
            (wo / 4096.0).reshape(2, 128, EMB).transpose(1, 0, 2)
            .reshape(128, 2048).astype(BF16NP))

        padb = np.where(padding_mask[b].reshape(NKC, 128).T.astype(bool),
                        0.0, NEGB).astype(np.float32)
        in_maps.append({
            "x8": x8, "wqk1": w1, "wqk2": w2, "wv1": wv1, "wv2": wv2,
            "wo": wo8, "padb": np.ascontiguousarray(padb), "ident": ident,
        })

    if np.any(b_qkv != 0):
        raise NotImplementedError("nonzero b_qkv not supported")
    nc = _get_nc()
    res = bass_utils.run_bass_kernel_spmd(nc, in_maps, core_ids=list(range(8)))
    out = np.zeros((B, S, EMB), np.float32)
    for c in range(8):
        out[c // 4] += res.results[c]["y"].astype(np.float32)
    out += b_o
    return out
